# revision 1
# baseline (speedup 1.0000x reference)
"""Trainium2 Bass kernel for nn_Attention_6794638262338.

Single-layer attention block with BitNet-style ternary-quantized projections:
    x -> LN1 -> qkv proj (ternary W) -> MHA softmax -> LN2 -> out proj (ternary W)

Strategy: pure data parallelism. batch=8, n_cores=8 -> one batch element per
core, no collectives. Each core runs an identical Bass/Tile program.

Math folds (host side):
  - ternary_quant(W) = T * s with T in {-1,0,1}: pass T in bf16 (exact), fold
    s_qkv^2 * DIM_HEAD^-0.5 into the exp() activation scale, fold s_qkv/s_out
    into the LN2 rsqrt epsilon/scale.
  - softmax denominator: out = (sum_m exp(s)*v) / colsum. colsum obtained free
    by appending a ones-column to v in the attn@v matmul (M=65); division done
    via DVE reciprocal + GpSimd partition_broadcast + DVE multiply.
  - LN2: mean/var via ones-matmul column sums of a^T, tiny PE transposes to get
    per-row stats, y = (z - mu*W1) * rsqrt-ish using host-precomputed
    W1 = rowsum of effective output weight.
"""

import numpy as np
from contextlib import ExitStack

import concourse.bass as bass
import concourse.mybir as mybir
import concourse.tile as tile
from concourse import bacc
from concourse.bass import ts, ds
from concourse.bass_utils import run_bass_kernel_spmd
from concourse.masks import make_identity

F32 = mybir.dt.float32
BF16 = mybir.dt.bfloat16
AF = mybir.ActivationFunctionType
ALU = mybir.AluOpType

B, N, D = 8, 1024, 512
H, DH = 8, 64
INNER = H * DH  # 512
NT = N // 128   # 8 n-tiles
DC = D // 128   # 4 d-chunks
EPS_LN = 1e-5
EPS_Q = 1e-6

TRACE = False          # set by test.py to capture an NTFF profile
LAST_RESULTS = None    # BassKernelResults of the most recent run

_CACHE = {}


def _ternary(w):
    """Replicate reference ternary_quant in fp32; return (unit ternary, scale)."""
    w = np.asarray(w, np.float32)
    s = np.float32(np.mean(np.abs(w), dtype=np.float32))
    t = np.round(np.clip(w / (s + np.float32(EPS_Q)), -1.0, 1.0)).astype(np.float32)
    return t, float(s)


def _emit(ctx: ExitStack, tc: "tile.TileContext", io: dict, c: dict, sfx: str = ""):
    nc = tc.nc
    dbg = c.get("debug", False)

    def dump(name, ap):
        if dbg:
            d = nc.dram_tensor(f"dbg_{name}{sfx}", list(ap.shape), ap.dtype, kind="ExternalOutput").ap()
            nc.sync.dma_start(out=d, in_=ap)
    x, tqT, toT, w1u, y = io["x"], io["tqT"], io["toT"], io["w1u"], io["y"]

    need_g1 = c["need_g1"]
    need_b1 = c["need_b1"]
    need_bt = c["need_bt"]

    # ---------------- pools ----------------
    const_p = ctx.enter_context(tc.tile_pool(name="const" + sfx, bufs=1))
    xp = ctx.enter_context(tc.tile_pool(name="xp" + sfx, bufs=3))
    lnp = ctx.enter_context(tc.tile_pool(name="lnp" + sfx, bufs=4))
    xlnp = ctx.enter_context(tc.tile_pool(name="xlnp" + sfx, bufs=3))
    big = ctx.enter_context(tc.tile_pool(name="big" + sfx, bufs=1))
    attp = ctx.enter_context(tc.tile_pool(name="attp" + sfx, bufs=2))
    smp = ctx.enter_context(tc.tile_pool(name="smp" + sfx, bufs=3))
    outp = ctx.enter_context(tc.tile_pool(name="outp" + sfx, bufs=2))
    # PSUM budget: 8 banks = ps_s ([128,1024] x2 = 4) + ps_o ([65,512] x2 = 2)
    #              + ps_m ([128,512] x2 = 2)
    ps_s = ctx.enter_context(tc.tile_pool(name="ps_s" + sfx, bufs=2, space="PSUM"))
    ps_o = ctx.enter_context(tc.tile_pool(name="ps_o" + sfx, bufs=2, space="PSUM"))
    ps_m = ctx.enter_context(tc.tile_pool(name="ps_m" + sfx, bufs=2, space="PSUM"))

    # ---------------- constants ----------------
    ident = const_p.tile([128, 128], BF16)
    make_identity(nc, ident)
    ones128 = const_p.tile([128, 1], BF16)
    nc.vector.memset(ones128, 1.0)
    eps1 = const_p.tile([128, 1], F32)
    nc.vector.memset(eps1, float(EPS_LN))
    eps2 = const_p.tile([128, 1], F32)
    nc.vector.memset(eps2, c["eps_eff"])
    # warm the ln/exp activation table while the first x tile is in flight
    warm = const_p.tile([128, 1], F32)
    nc.scalar.activation(warm, eps1, AF.Ln, bias=eps1)
    nc.scalar.activation(warm, warm, AF.Exp, scale=-0.5)

    # qkv unit-ternary weights, transposed: [d, 3*inner] -> sbuf [128, DC, 3*inner]
    tq_sb = const_p.tile([128, DC, 3 * INNER], BF16)
    nc.sync.dma_start(out=tq_sb, in_=tqT.rearrange("(c p) o -> p c o", p=128))
    # out-proj unit weights (g2 folded), transposed: [o, dout] -> [128, DC, dout]
    toT_sb = const_p.tile([128, DC, INNER], BF16)
    nc.sync.dma_start(out=toT_sb, in_=toT.rearrange("(c p) o -> p c o", p=128))
    # W1 rowsums broadcast across partitions
    w1b = const_p.tile([128, INNER], F32)
    nc.gpsimd.dma_start(
        out=w1b,
        in_=bass.AP(tensor=w1u.tensor, offset=w1u.offset, ap=[[0, 128]] + list(w1u.ap)),
    )
    if need_g1:
        g1_ap = io["g1v"]
        g1b = const_p.tile([128, D], F32)
        nc.gpsimd.dma_start(
            out=g1b,
            in_=bass.AP(tensor=g1_ap.tensor, offset=g1_ap.offset, ap=[[0, 128]] + list(g1_ap.ap)),
        )
    if need_b1:
        b1_ap = io["b1v"]
        b1b = const_p.tile([128, D], F32)
        nc.gpsimd.dma_start(
            out=b1b,
            in_=bass.AP(tensor=b1_ap.tensor, offset=b1_ap.offset, ap=[[0, 128]] + list(b1_ap.ap)),
        )
    if need_bt:
        bt_ap = io["btv"]
        btb = const_p.tile([128, INNER], F32)
        nc.gpsimd.dma_start(
            out=btb,
            in_=bass.AP(tensor=bt_ap.tensor, offset=bt_ap.offset, ap=[[0, 128]] + list(bt_ap.ap)),
        )

    # ---------------- persistent big tensors ----------------
    # xln^T: [d, n] bf16 as [128, DC, N]   (partition = d within chunk)
    xlnT = big.tile([128, DC, N], BF16)
    # q^T, k^T head-major: [o, n] as [128, DC, N] (o = otile*128 + p)
    qT = big.tile([128, DC, N], BF16)
    kT = big.tile([128, DC, N], BF16)
    # v row-major with ones column: [128, mt, h, 65] (m = mt*128 + p)
    v_sb = big.tile([128, NT, H, DH + 1], BF16)
    nc.vector.memset(v_sb[:, :, :, DH : DH + 1], 1.0)
    # staging for odd heads' divided output (pre partition-remap)
    aT = big.tile([64, DC, N], BF16)
    # pair-stacked repack of aT for K=128 matmuls: partition 0:64 = head 2p,
    # 64:128 = head 2p+1 (built by SBUF->SBUF DMA partition remap)
    aT2 = big.tile([128, DC, N], BF16)
    # colsum reciprocal staging at partition 64
    rc64 = big.tile([65, 2, 512], F32)
    # squares of aT2 for the LN2 sum-of-squares (filled by GpSimd)
    sq_sb = big.tile([128, DC, N], BF16)

    # ================ Phase A: load x, LN1, transpose ================
    for nt in range(NT):
        xt = xp.tile([128, D], F32, name="xt", tag="xt")
        nc.sync.dma_start(out=xt, in_=x[ts(nt, 128), :])
        st6 = lnp.tile([128, 6], F32, name="st6", tag="st6")
        nc.vector.bn_stats(st6, xt)
        mv = lnp.tile([128, 2], F32, name="mv", tag="mv")
        nc.vector.bn_aggr(mv, st6)
        # rstd = exp(-0.5*ln(var+eps)) — keeps ACT on the ln/exp table set
        # (same set the attention exp uses; avoids sqrt-set thrashing)
        sd = lnp.tile([128, 1], F32, name="sd", tag="sd")
        nc.scalar.activation(sd, mv[:, 1:2], AF.Ln, bias=eps1)
        rs = lnp.tile([128, 1], F32, name="rs", tag="rs")
        nc.scalar.activation(rs, sd, AF.Exp, scale=-0.5)
        xl = xlnp.tile([128, D], BF16, name="xl", tag="xl")
        if need_g1 or need_b1:
            xlf = xlnp.tile([128, D], F32, name="xlf", tag="xlf")
            nc.vector.tensor_scalar(
                out=xlf, in0=xt, scalar1=mv[:, 0:1], scalar2=rs,
                op0=ALU.subtract, op1=ALU.mult,
            )
            if need_g1:
                nc.vector.tensor_mul(xlf, xlf, g1b)
            if need_b1:
                nc.vector.tensor_add(xlf, xlf, b1b)
            nc.vector.tensor_copy(xl, xlf)
        else:
            nc.vector.tensor_scalar(
                out=xl, in0=xt, scalar1=mv[:, 0:1], scalar2=rs,
                op0=ALU.subtract, op1=ALU.mult,
            )
        # transpose via matmul with identity: out = xl_slice.T. All four
        # d-chunks land in one psum tile -> one strided copy into xlnT.
        pt = ps_m.tile([128, DC, 128], F32, name="pt", tag="mm")
        for dc in range(DC):
            nc.tensor.matmul(
                pt[:, dc, :], lhsT=xl[:, ts(dc, 128)], rhs=ident, start=True, stop=True
            )
        nc.vector.tensor_copy(out=xlnT[:, :, ts(nt, 128)], in_=pt)

    dump("xlnT", xlnT)

    # ================ Phase B+C interleaved: qkv otiles feed attention
    # head-pairs as soon as their q/k tile is ready, so ACT starts exp()
    # early and stays the pacer without idle lead-in. ================
    def emit_qk(ot):
        # q, k head-major: psum[o_tile, n] = sum_dc Tq[:,dc,ot].T @ xlnT[:,dc,n]
        # (qkv psums live in ps_m so the scores pool slots stay dedicated to
        # the ACT exp pipeline)
        for sec, dst in ((0, qT), (1, kT)):
            for nn in range(2):
                pq = ps_m.tile([128, 512], F32, name="pq", tag="mm")
                for dc in range(DC):
                    nc.tensor.matmul(
                        pq,
                        lhsT=tq_sb[:, dc, ds(sec * INNER + ot * 128, 128)],
                        rhs=xlnT[:, dc, ts(nn, 512)],
                        start=(dc == 0), stop=(dc == DC - 1),
                    )
                nc.vector.tensor_copy(out=dst[:, ot, ts(nn, 512)], in_=pq)

    def emit_v():
        # v row-major: psum[m_tile, o] = sum_dc xlnT[:,dc,mt].T @ Tq_v[:,dc,:]
        for mt in range(NT):
            pv = ps_m.tile([128, 512], F32, name="pv", tag="mm")
            for dc in range(DC):
                nc.tensor.matmul(
                    pv,
                    lhsT=xlnT[:, dc, ts(mt, 128)],
                    rhs=tq_sb[:, dc, ds(2 * INNER, INNER)],
                    start=(dc == 0), stop=(dc == DC - 1),
                )
            # strided copy into per-head layout [128, h, 64]
            nc.vector.tensor_copy(
                out=v_sb[:, mt, :, 0:DH],
                in_=pv.rearrange("p (h d) -> p h d", h=H),
            )

    cs_dram = nc.dram_tensor("cs_scratch" + sfx, [H, 2, 512], F32).ap()
    scale_exp = c["scale_exp"]

    def emit_scores_pair(p):
        """Scores+exp for heads 2p (partitions 0:64) and 2p+1 (64:128).
        The two heads' K=64 matmuls land on disjoint PE row groups
        (tile_position auto-derived from base partition) and overlap."""
        atns = []
        for sub in range(2):
            atns.append(attp.tile([128, NT, N], BF16, name=f"atn{sub}", tag=f"atn{sub}"))
        for mt in range(NT):
            pss = [
                ps_s.tile([128, N], F32, name="pssa", tag="s"),
                ps_s.tile([128, N], F32, name="pssb", tag="s"),
            ]
            for nn in range(2):
                for sub in range(2):
                    base = sub * 64
                    nc.tensor.matmul(
                        pss[sub][:, ts(nn, 512)],
                        lhsT=kT[ds(base, 64), p, ts(mt, 128)],
                        rhs=qT[ds(base, 64), p, ts(nn, 512)],
                        start=True, stop=True,
                    )
            for sub in range(2):
                nc.scalar.activation(
                    out=atns[sub][:, mt, :], in_=pss[sub], func=AF.Exp, scale=scale_exp
                )
        return atns

    def emit_out(h, atn):
        po2 = [
            ps_o.tile([65, 512], F32, name="po0", tag="po"),
            ps_o.tile([65, 512], F32, name="po1", tag="po"),
        ]
        for mt in range(NT):
            for nn in range(2):
                nc.tensor.matmul(
                    po2[nn],
                    lhsT=v_sb[:, mt, h, :],
                    rhs=atn[:, mt, ts(nn, 512)],
                    start=(mt == 0), stop=(mt == NT - 1),
                )
        stg = smp.tile([65, 2, 512], F32, name="stg", tag="stg")
        for nn in range(2):
            # stage PSUM out to SBUF immediately so the accumulator slot
            # frees for the next head; the slow divide chain (reciprocal ->
            # DRAM-bounce partition broadcast -> multiply) runs off SBUF.
            nc.vector.tensor_copy(stg[:, nn, :], po2[nn])
        for nn in range(2):
            nc.vector.reciprocal(rc64[64:65, nn, :], stg[64:65, nn, :])
            nc.sync.dma_start(out=cs_dram[h, nn, :], in_=rc64[64:65, nn, :])
            rbt = smp.tile([64, 512], F32, name="rbt", tag="rbt")
            src = cs_dram[h, nn, :]
            nc.sync.dma_start(
                out=rbt,
                in_=bass.AP(tensor=src.tensor, offset=src.offset,
                            ap=[[0, 64]] + list(src.ap)),
            )
            # even heads land on partitions 0:64 of their aT2 pair-chunk
            # directly; odd heads stage in aT then partition-remap via DMA
            div_dst = (
                aT2[ds(0, 64), h // 2, ts(nn, 512)]
                if h % 2 == 0
                else aT[:, h // 2, ts(nn, 512)]
            )
            nc.vector.tensor_tensor(
                out=div_dst, in0=stg[0:64, nn, :], in1=rbt, op=ALU.mult,
            )
            if h == 0 and dbg:
                dump(f"po_h0_n{nn}", stg[:, nn, :])
                dump(f"rc64_h0_n{nn}", rc64[64:65, nn, :])
                dump(f"rbt_h0_n{nn}", rbt)
        if h == 0:
            dump("atn_h0", atn)
        if h % 2 == 1:
            nc.sync.dma_start(out=aT2[ds(64, 64), h // 2, :], in_=aT[:, h // 2, :])

    # driver: scores-pair 0 starts as soon as its q/k tile exists (ACT
    # starts exp'ing early); v and the next pair's q/k are emitted behind
    # the current pair's scores so PE fills its exp-wait slack with them;
    # out-matmuls run one pair behind. Squares for the LN2 sum-of-squares
    # run on idle GpSimd as chunks finish (last chunk on DVE: tail-critical).
    emit_qk(0)
    prev = emit_scores_pair(0)
    emit_v()
    emit_qk(1)
    for pair in range(1, 4):
        atns = emit_scores_pair(pair)
        if pair < 3:
            emit_qk(pair + 1)
        pp = pair - 1
        emit_out(2 * pp, prev[0])
        emit_out(2 * pp + 1, prev[1])
        nc.gpsimd.tensor_mul(sq_sb[:, pp, :], aT2[:, pp, :], aT2[:, pp, :])
        prev = atns
    emit_out(6, prev[0])
    emit_out(7, prev[1])
    nc.vector.tensor_mul(sq_sb[:, 3, :], aT2[:, 3, :], aT2[:, 3, :])

    dump("qT", qT)
    dump("kT", kT)
    dump("v", v_sb)
    dump("aT2", aT2)

    # ================ Phase D: LN2 stats + output projection ================
    # z[n,dout] = sum_o a[n,o]*toT[o,dout] per n-tile; the LN2 row sums
    # s1[n] = sum_o a, s2[n] = sum_o a^2 come out n-major (as per-partition
    # columns) from N=1 matmuls sharing/reusing the same stationary chunks.
    s1col = ps_o.tile([128, NT], F32, name="s1col", tag="po")
    s2col = ps_o.tile([128, NT], F32, name="s2col", tag="po")
    z_sb = big.tile([128, NT, INNER], BF16)
    for nt in range(NT):
        pz = ps_m.tile([128, INNER], F32, name="pz", tag="mm")
        for ch in range(DC):
            nc.tensor.matmul(
                pz, lhsT=aT2[:, ch, ts(nt, 128)], rhs=toT_sb[:, ch, :],
                start=(ch == 0), stop=(ch == DC - 1),
            )
            nc.tensor.matmul(
                s1col[:, nt : nt + 1], lhsT=aT2[:, ch, ts(nt, 128)], rhs=ones128,
                start=(ch == 0), stop=(ch == DC - 1),
            )
        for ch in range(DC):
            nc.tensor.matmul(
                s2col[:, nt : nt + 1], lhsT=sq_sb[:, ch, ts(nt, 128)], rhs=ones128,
                start=(ch == 0), stop=(ch == DC - 1),
            )
        nc.vector.tensor_copy(z_sb[:, nt, :], pz)

    # mu = s1/512 ; var = s2/512 - mu^2 ; r2 = s_o / sqrt(var + eps_eff)
    mu = lnp.tile([128, NT], F32, name="mu", tag="mu", bufs=1)
    nc.scalar.mul(mu, s1col, 1.0 / INNER)
    es = lnp.tile([128, NT], F32, name="es", tag="es", bufs=1)
    nc.scalar.mul(es, s2col, 1.0 / INNER)
    musq = lnp.tile([128, NT], F32, name="musq", tag="musq", bufs=1)
    nc.vector.tensor_mul(musq, mu, mu)
    var = lnp.tile([128, NT], F32, name="var", tag="var", bufs=1)
    nc.vector.tensor_sub(var, es, musq)
    sd2 = lnp.tile([128, NT], F32, name="sd2", tag="sd2", bufs=1)
    nc.scalar.activation(sd2, var, AF.Ln, bias=eps2, scale=c["inv_so2"])
    r2 = lnp.tile([128, NT], F32, name="r2", tag="r2", bufs=1)
    nc.scalar.activation(r2, sd2, AF.Exp, scale=-0.5)
    r2n = lnp.tile([128, NT], F32, name="r2n", tag="r2n", bufs=1)
    nc.vector.tensor_scalar_mul(r2n, r2, -1.0)
    dump("mu", mu)
    dump("r2", r2)

    # y = (z - mu*W1) * r2 (+ bias_total), fused as u = (W1*mu) - z ; y = u*(-r2)
    for nt in range(NT):
        yt = outp.tile([128, INNER], F32, name="yt", tag="yt")
        nc.vector.scalar_tensor_tensor(
            out=yt, in0=w1b, scalar=mu[:, nt : nt + 1], in1=z_sb[:, nt, :],
            op0=ALU.mult, op1=ALU.subtract,
        )
        nc.vector.tensor_scalar_mul(yt, yt, r2n[:, nt : nt + 1])
        if need_bt:
            nc.vector.tensor_add(yt, yt, btb)
        nc.sync.dma_start(out=y[ts(nt, 128), :], in_=yt)


def _build(c: dict):
    nc = bacc.Bacc("TRN2", target_bir_lowering=False, debug=False, num_devices=B)
    io = {
        "x": nc.dram_tensor("x", [N, D], F32, kind="ExternalInput").ap(),
        "tqT": nc.dram_tensor("tqT", [D, 3 * INNER], BF16, kind="ExternalInput").ap(),
        "toT": nc.dram_tensor("toT", [INNER, INNER], BF16, kind="ExternalInput").ap(),
        "w1u": nc.dram_tensor("w1u", [INNER], F32, kind="ExternalInput").ap(),
        "y": nc.dram_tensor("y", [N, D], F32, kind="ExternalOutput").ap(),
    }
    if c["need_g1"]:
        io["g1v"] = nc.dram_tensor("g1v", [D], F32, kind="ExternalInput").ap()
    if c["need_b1"]:
        io["b1v"] = nc.dram_tensor("b1v", [D], F32, kind="ExternalInput").ap()
    if c["need_bt"]:
        io["btv"] = nc.dram_tensor("btv", [INNER], F32, kind="ExternalInput").ap()
    reps = c.get("body_reps", 1)
    with tile.TileContext(nc) as tc:
        for r in range(reps):
            with ExitStack() as ctx:
                _emit(ctx, tc, io, c, sfx="" if r == 0 else f"_r{r}")

    nc.compile()

    # The act-table-load pass greedily picks the first set containing each
    # function, thrashing between `natural_log` (Ln) and `exp_and_others`
    # (Exp) on every rstd computation (18 reloads @ ~1.3-2.7us each). All
    # activation funcs this kernel uses (Ln, Exp, Copy, Identity) live
    # together in `natural_log_exp_and_others`, so rewrite the first load to
    # that set and drop the rest.
    from concourse.hw_specs import get_activation_tables
    tset = list(get_activation_tables(nc.m.arch).keys())
    nle = tset.index("natural_log_exp_and_others")
    for blk in nc.main_func.blocks:
        keep, first = [], False
        for inst in blk.instructions:
            if type(inst).__name__ == "InstLoadActFuncSet":
                si = getattr(inst, "sync_info", None)
                clean = si is None or (not si.on_wait and not si.on_update)
                if not first:
                    inst.act_func_set_id = nle
                    first = True
                    keep.append(inst)
                elif not clean:
                    inst.act_func_set_id = nle
                    keep.append(inst)
            else:
                keep.append(inst)
        blk.instructions[:] = keep
    return nc


def _prep(inputs):
    g1 = np.asarray(inputs["g1"], np.float32)
    b1 = np.asarray(inputs["b1"], np.float32)
    g2 = np.asarray(inputs["g2"], np.float32)
    b2 = np.asarray(inputs["b2"], np.float32)
    b_out = np.asarray(inputs["b_out"], np.float32)

    Tq, s_q = _ternary(inputs["W_qkv"])   # [3*inner, d]
    To, s_o = _ternary(inputs["W_out"])   # [dout, o]

    Wp = To * g2[None, :]                 # fold g2 (exact when g2 == 1)
    toT = np.ascontiguousarray(Wp.T)      # [o, dout]
    w1u = Wp.sum(axis=1).astype(np.float32)
    bias_total = (b2 @ To.T) * np.float32(s_o) + b_out

    c = {
        "scale_exp": float(s_q * s_q * (DH ** -0.5)),
        "inv_so2": float(1.0 / (s_o * s_o)),
        "eps_eff": float(EPS_LN / (s_q * s_q * s_o * s_o)),
        "need_g1": bool(not np.allclose(g1, 1.0)),
        "need_b1": bool(np.any(b1)),
        "need_bt": bool(np.any(bias_total)),
    }
    arrs = {
        "tqT": np.ascontiguousarray(Tq.T),
        "toT": toT,
        "w1u": w1u,
        "g1": g1, "b1": b1, "bt": bias_total,
    }
    return c, arrs


def _to_bf16(a):
    import ml_dtypes
    return np.asarray(a, np.float32).astype(ml_dtypes.bfloat16)


def kernel(**inputs) -> np.ndarray:
    global LAST_RESULTS
    x = np.asarray(inputs["x"], np.float32)
    assert x.shape == (B, N, D)
    c, arrs = _prep(inputs)

    key = tuple(sorted(c.items()))
    if key not in _CACHE:
        _CACHE[key] = _build(c)
    nc = _CACHE[key]

    base = {
        "tqT": _to_bf16(arrs["tqT"]),
        "toT": _to_bf16(arrs["toT"]),
        "w1u": arrs["w1u"].astype(np.float32),
    }
    if c["need_g1"]:
        base["g1v"] = arrs["g1"]
    if c["need_b1"]:
        base["b1v"] = arrs["b1"]
    if c["need_bt"]:
        base["btv"] = arrs["bt"].astype(np.float32)

    in_maps = [dict(base, x=np.ascontiguousarray(x[i])) for i in range(B)]
    res = run_bass_kernel_spmd(nc, in_maps, core_ids=list(range(B)), trace=TRACE)
    LAST_RESULTS = res
    out = np.stack([res.results[i]["y"] for i in range(B)], axis=0)
    return out.astype(np.float32)


def bench_exec_ns(inputs, iters=32, reps=5, body_reps=1):
    """Measure per-execution NEFF time by chaining `iters` sequential
    executions inside one jitted program (chained through the output
    buffers) and comparing against a 1-execution program."""
    import time as _time
    import jax
    from jax.experimental.shard_map import shard_map
    from jax.sharding import Mesh, PartitionSpec, NamedSharding
    from concourse import bass2jax, mybir as _mybir

    x = np.asarray(inputs["x"], np.float32)
    c, arrs = _prep(inputs)
    if body_reps != 1:
        c["body_reps"] = body_reps
    key = tuple(sorted(c.items()))
    if key not in _CACHE:
        _CACHE[key] = _build(c)
    nc = _CACHE[key]
    bass2jax.install_neuronx_cc_hook()

    base = {
        "tqT": _to_bf16(arrs["tqT"]),
        "toT": _to_bf16(arrs["toT"]),
        "w1u": arrs["w1u"].astype(np.float32),
    }
    if c["need_g1"]:
        base["g1v"] = arrs["g1"]
    if c["need_b1"]:
        base["b1v"] = arrs["b1"]
    if c["need_bt"]:
        base["btv"] = arrs["bt"].astype(np.float32)
    in_maps = [dict(base, x=np.ascontiguousarray(x[i])) for i in range(B)]

    partition_name = nc.partition_id_tensor.name if nc.partition_id_tensor else None
    in_names, out_names, out_avals, zero_outs = [], [], [], []
    for alloc in nc.m.functions[0].allocations:
        if not isinstance(alloc, mybir.MemoryLocationSet):
            continue
        name = alloc.memorylocations[0].name
        if alloc.kind == "ExternalInput":
            if name != partition_name:
                in_names.append(name)
        elif alloc.kind == "ExternalOutput":
            out_names.append(name)
            shape = tuple(alloc.tensor_shape)
            dtype = mybir.dt.np(alloc.dtype)
            out_avals.append(jax.core.ShapedArray(shape, dtype))
            zero_outs.append(np.zeros(shape, dtype))
    n_params = len(in_names)

    bind_names = list(in_names) + list(out_names)
    if partition_name is not None:
        bind_names.append(partition_name)

    def _body(*args):
        operands = list(args)
        pid = [bass2jax.partition_id_tensor()] if partition_name else []
        outs = bass2jax._bass_exec_p.bind(
            *(operands + pid),
            out_avals=tuple(out_avals),
            in_names=tuple(bind_names),
            out_names=tuple(out_names),
            lowering_input_output_aliases=(),
            sim_require_finite=True,
            sim_require_nnan=True,
            nc=nc,
        )
        return tuple(outs)

    devices = jax.devices()[:B]
    mesh = Mesh(np.asarray(devices), ("core",))
    spec = PartitionSpec("core")
    n_out = len(out_names)
    per_core = [[np.asarray(m[nm]) for nm in in_names] for m in in_maps]
    concat_in = [
        np.concatenate([per_core[cc][i] for cc in range(B)], axis=0)
        for i in range(n_params)
    ]
    concat_zeros = [
        np.zeros((B * z.shape[0], *z.shape[1:]), z.dtype) for z in zero_outs
    ]
    dev_args = [
        jax.device_put(a, NamedSharding(mesh, spec)) for a in concat_in + concat_zeros
    ]

    f = jax.jit(
        shard_map(
            _body, mesh=mesh,
            in_specs=(spec,) * (n_params + n_out),
            out_specs=(spec,) * n_out,
            check_rep=False,
        )
    )
    jax.block_until_ready(f(*dev_args))  # compile + warm

    times = {}
    for k in (1, iters):
        best = float("inf")
        for _ in range(reps):
            t0 = _time.perf_counter()
            r = None
            for _ in range(k):
                r = f(*dev_args)  # async dispatch; device executes in-order
            jax.block_until_ready(r)
            best = min(best, _time.perf_counter() - t0)
        times[k] = best
    exec_ns = (times[iters] - times[1]) / (iters - 1) * 1e9
    return exec_ns, times



# revision 47
# speedup vs baseline: 5.8637x; 5.8637x over previous
"""Trainium2 Bass kernel for nn_Attention_6794638262338.

Single-layer attention block with BitNet-style ternary-quantized projections:
    x -> LN1 -> qkv proj (ternary W) -> MHA softmax -> LN2 -> out proj (ternary W)

Strategy: pure data parallelism. batch=8, n_cores=8 -> one batch element per
core, no collectives. Each core runs an identical Bass/Tile program.

Math folds (host side):
  - ternary_quant(W) = T * s with T in {-1,0,1}: pass T in bf16 (exact), fold
    s_qkv^2 * DIM_HEAD^-0.5 into the exp() activation scale, fold s_qkv/s_out
    into the LN2 rsqrt epsilon/scale.
  - softmax denominator: out = (sum_m exp(s)*v) / colsum. colsum obtained free
    by appending a ones-column to v in the attn@v matmul (M=65); division done
    via DVE reciprocal + GpSimd partition_broadcast + DVE multiply.
  - LN2: mean/var via ones-matmul column sums of a^T, tiny PE transposes to get
    per-row stats, y = (z - mu*W1) * rsqrt-ish using host-precomputed
    W1 = rowsum of effective output weight.
"""

import numpy as np
from contextlib import ExitStack

import concourse.bass as bass
import concourse.mybir as mybir
import concourse.tile as tile
from concourse import bacc
from concourse.bass import ts, ds
from concourse.bass_utils import run_bass_kernel_spmd
from concourse.masks import make_identity

F32 = mybir.dt.float32
BF16 = mybir.dt.bfloat16
AF = mybir.ActivationFunctionType
ALU = mybir.AluOpType

B, N, D = 8, 1024, 512
H, DH = 8, 64
INNER = H * DH  # 512
NT = N // 128   # 8 n-tiles
DC = D // 128   # 4 d-chunks
EPS_LN = 1e-5
EPS_Q = 1e-6

TRACE = False          # set by test.py to capture an NTFF profile
LAST_RESULTS = None    # BassKernelResults of the most recent run

_CACHE = {}


def _ternary(w):
    """Replicate reference ternary_quant in fp32; return (unit ternary, scale)."""
    w = np.asarray(w, np.float32)
    s = np.float32(np.mean(np.abs(w), dtype=np.float32))
    t = np.round(np.clip(w / (s + np.float32(EPS_Q)), -1.0, 1.0)).astype(np.float32)
    return t, float(s)


def _emit(ctx: ExitStack, tc: "tile.TileContext", io: dict, c: dict, sfx: str = ""):
    nc = tc.nc
    dbg = c.get("debug", False)

    def dump(name, ap):
        if dbg:
            d = nc.dram_tensor(f"dbg_{name}{sfx}", list(ap.shape), ap.dtype, kind="ExternalOutput").ap()
            nc.sync.dma_start(out=d, in_=ap)
    x, tqT, toT, w1u, y = io["x"], io["tqT"], io["toT"], io["w1u"], io["y"]

    need_g1 = c["need_g1"]
    need_b1 = c["need_b1"]
    need_bt = c["need_bt"]

    # ---------------- pools ----------------
    const_p = ctx.enter_context(tc.tile_pool(name="const" + sfx, bufs=1))
    xp = ctx.enter_context(tc.tile_pool(name="xp" + sfx, bufs=3))
    lnp = ctx.enter_context(tc.tile_pool(name="lnp" + sfx, bufs=4))
    xlnp = ctx.enter_context(tc.tile_pool(name="xlnp" + sfx, bufs=3))
    big = ctx.enter_context(tc.tile_pool(name="big" + sfx, bufs=1))
    attp = ctx.enter_context(tc.tile_pool(name="attp" + sfx, bufs=2))
    smp = ctx.enter_context(tc.tile_pool(name="smp" + sfx, bufs=3))
    outp = ctx.enter_context(tc.tile_pool(name="outp" + sfx, bufs=2))
    # PSUM budget: 8 banks = ps_s ([128,1024] x2 = 4) + ps_o ([65,512] x2 = 2)
    #              + ps_m ([128,512] x2 = 2)
    ps_s = ctx.enter_context(tc.tile_pool(name="ps_s" + sfx, bufs=2, space="PSUM"))
    ps_o = ctx.enter_context(tc.tile_pool(name="ps_o" + sfx, bufs=2, space="PSUM"))
    ps_m = ctx.enter_context(tc.tile_pool(name="ps_m" + sfx, bufs=2, space="PSUM"))

    # ---------------- constants ----------------
    ident = const_p.tile([128, 128], BF16)
    make_identity(nc, ident)
    ones128 = const_p.tile([128, 1], BF16)
    nc.vector.memset(ones128, 1.0)
    eps1 = const_p.tile([128, 1], F32)
    nc.vector.memset(eps1, float(EPS_LN))
    eps2 = const_p.tile([128, 1], F32)
    nc.vector.memset(eps2, c["eps_eff"])
    # warm the ln/exp activation table while the first x tile is in flight
    warm = const_p.tile([128, 1], F32)
    nc.scalar.activation(warm, eps1, AF.Ln, bias=eps1)
    nc.scalar.activation(warm, warm, AF.Exp, scale=-0.5)

    # qkv unit-ternary weights, transposed: [d, 3*inner] -> sbuf [128, DC, 3*inner]
    tq_sb = const_p.tile([128, DC, 3 * INNER], BF16)
    nc.sync.dma_start(out=tq_sb, in_=tqT.rearrange("(c p) o -> p c o", p=128))
    # out-proj unit weights (g2 folded), transposed: [o, dout] -> [128, DC, dout]
    toT_sb = const_p.tile([128, DC, INNER], BF16)
    nc.sync.dma_start(out=toT_sb, in_=toT.rearrange("(c p) o -> p c o", p=128))
    # W1 rowsums broadcast across partitions
    w1b = const_p.tile([128, INNER], F32)
    nc.gpsimd.dma_start(
        out=w1b,
        in_=bass.AP(tensor=w1u.tensor, offset=w1u.offset, ap=[[0, 128]] + list(w1u.ap)),
    )
    if need_g1:
        g1_ap = io["g1v"]
        g1b = const_p.tile([128, D], F32)
        nc.gpsimd.dma_start(
            out=g1b,
            in_=bass.AP(tensor=g1_ap.tensor, offset=g1_ap.offset, ap=[[0, 128]] + list(g1_ap.ap)),
        )
    if need_b1:
        b1_ap = io["b1v"]
        b1b = const_p.tile([128, D], F32)
        nc.gpsimd.dma_start(
            out=b1b,
            in_=bass.AP(tensor=b1_ap.tensor, offset=b1_ap.offset, ap=[[0, 128]] + list(b1_ap.ap)),
        )
    if need_bt:
        bt_ap = io["btv"]
        btb = const_p.tile([128, INNER], F32)
        nc.gpsimd.dma_start(
            out=btb,
            in_=bass.AP(tensor=bt_ap.tensor, offset=bt_ap.offset, ap=[[0, 128]] + list(bt_ap.ap)),
        )

    # ---------------- persistent big tensors ----------------
    # xln^T: [d, n] bf16 as [128, DC, N]   (partition = d within chunk)
    xlnT = big.tile([128, DC, N], BF16)
    # q^T, k^T head-major: [o, n] as [128, DC, N] (o = otile*128 + p)
    qT = big.tile([128, DC, N], BF16)
    kT = big.tile([128, DC, N], BF16)
    # v row-major with ones column: [128, mt, h, 65] (m = mt*128 + p)
    v_sb = big.tile([128, NT, H, DH + 1], BF16)
    nc.vector.memset(v_sb[:, :, :, DH : DH + 1], 1.0)
    # staging for odd heads' divided output (pre partition-remap)
    aT = big.tile([64, DC, N], BF16)
    # pair-stacked repack of aT for K=128 matmuls: partition 0:64 = head 2p,
    # 64:128 = head 2p+1 (built by SBUF->SBUF DMA partition remap)
    aT2 = big.tile([128, DC, N], BF16)
    # colsum reciprocal staging at partition 64
    rc64 = big.tile([65, 2, 512], F32)
    # squares of aT2 for the LN2 sum-of-squares (filled by GpSimd)
    sq_sb = big.tile([128, DC, N], BF16)

    # ================ Phase A: load x, LN1, transpose ================
    for nt in range(NT):
        xt = xp.tile([128, D], F32, name="xt", tag="xt")
        nc.sync.dma_start(out=xt, in_=x[ts(nt, 128), :])
        st6 = lnp.tile([128, 6], F32, name="st6", tag="st6")
        nc.vector.bn_stats(st6, xt)
        mv = lnp.tile([128, 2], F32, name="mv", tag="mv")
        nc.vector.bn_aggr(mv, st6)
        # rstd = exp(-0.5*ln(var+eps)) — keeps ACT on the ln/exp table set
        # (same set the attention exp uses; avoids sqrt-set thrashing)
        sd = lnp.tile([128, 1], F32, name="sd", tag="sd")
        nc.scalar.activation(sd, mv[:, 1:2], AF.Ln, bias=eps1)
        rs = lnp.tile([128, 1], F32, name="rs", tag="rs")
        nc.scalar.activation(rs, sd, AF.Exp, scale=-0.5)
        xl = xlnp.tile([128, D], BF16, name="xl", tag="xl")
        if need_g1 or need_b1:
            xlf = xlnp.tile([128, D], F32, name="xlf", tag="xlf")
            nc.vector.tensor_scalar(
                out=xlf, in0=xt, scalar1=mv[:, 0:1], scalar2=rs,
                op0=ALU.subtract, op1=ALU.mult,
            )
            if need_g1:
                nc.vector.tensor_mul(xlf, xlf, g1b)
            if need_b1:
                nc.vector.tensor_add(xlf, xlf, b1b)
            nc.vector.tensor_copy(xl, xlf)
        else:
            nc.vector.tensor_scalar(
                out=xl, in0=xt, scalar1=mv[:, 0:1], scalar2=rs,
                op0=ALU.subtract, op1=ALU.mult,
            )
        # transpose via matmul with identity: out = xl_slice.T. All four
        # d-chunks land in one psum tile -> one strided copy into xlnT.
        pt = ps_m.tile([128, DC, 128], F32, name="pt", tag="mm")
        for dc in range(DC):
            nc.tensor.matmul(
                pt[:, dc, :], lhsT=xl[:, ts(dc, 128)], rhs=ident, start=True, stop=True
            )
        nc.vector.tensor_copy(out=xlnT[:, :, ts(nt, 128)], in_=pt)

    dump("xlnT", xlnT)

    # ================ Phase B+C interleaved: qkv otiles feed attention
    # head-pairs as soon as their q/k tile is ready, so ACT starts exp()
    # early and stays the pacer without idle lead-in. ================
    def emit_qk(ot):
        # q, k head-major: psum[o_tile, n] = sum_dc Tq[:,dc,ot].T @ xlnT[:,dc,n]
        # (qkv psums live in ps_m so the scores pool slots stay dedicated to
        # the ACT exp pipeline)
        for sec, dst in ((0, qT), (1, kT)):
            for nn in range(2):
                pq = ps_m.tile([128, 512], F32, name="pq", tag="mm")
                for dc in range(DC):
                    nc.tensor.matmul(
                        pq,
                        lhsT=tq_sb[:, dc, ds(sec * INNER + ot * 128, 128)],
                        rhs=xlnT[:, dc, ts(nn, 512)],
                        start=(dc == 0), stop=(dc == DC - 1),
                    )
                nc.vector.tensor_copy(out=dst[:, ot, ts(nn, 512)], in_=pq)

    def emit_v():
        # v row-major: psum[m_tile, o] = sum_dc xlnT[:,dc,mt].T @ Tq_v[:,dc,:]
        for mt in range(NT):
            pv = ps_m.tile([128, 512], F32, name="pv", tag="mm")
            for dc in range(DC):
                nc.tensor.matmul(
                    pv,
                    lhsT=xlnT[:, dc, ts(mt, 128)],
                    rhs=tq_sb[:, dc, ds(2 * INNER, INNER)],
                    start=(dc == 0), stop=(dc == DC - 1),
                )
            # strided copy into per-head layout [128, h, 64]
            nc.vector.tensor_copy(
                out=v_sb[:, mt, :, 0:DH],
                in_=pv.rearrange("p (h d) -> p h d", h=H),
            )

    cs_dram = nc.dram_tensor("cs_scratch" + sfx, [H, 2, 512], F32).ap()
    scale_exp = c["scale_exp"]

    def emit_scores_pair(p):
        """Scores+exp for heads 2p (partitions 0:64) and 2p+1 (64:128).
        The two heads' K=64 matmuls land on disjoint PE row groups
        (tile_position auto-derived from base partition) and overlap."""
        atns = []
        for sub in range(2):
            atns.append(attp.tile([128, NT, N], BF16, name=f"atn{sub}", tag=f"atn{sub}"))
        for mt in range(NT):
            pss = [
                ps_s.tile([128, N], F32, name="pssa", tag="s"),
                ps_s.tile([128, N], F32, name="pssb", tag="s"),
            ]
            for nn in range(2):
                for sub in range(2):
                    base = sub * 64
                    nc.tensor.matmul(
                        pss[sub][:, ts(nn, 512)],
                        lhsT=kT[ds(base, 64), p, ts(mt, 128)],
                        rhs=qT[ds(base, 64), p, ts(nn, 512)],
                        start=True, stop=True,
                    )
            for sub in range(2):
                nc.scalar.activation(
                    out=atns[sub][:, mt, :], in_=pss[sub], func=AF.Exp, scale=scale_exp
                )
        return atns

    def emit_out(h, atn):
        po2 = [
            ps_o.tile([65, 512], F32, name="po0", tag="po"),
            ps_o.tile([65, 512], F32, name="po1", tag="po"),
        ]
        for mt in range(NT):
            for nn in range(2):
                nc.tensor.matmul(
                    po2[nn],
                    lhsT=v_sb[:, mt, h, :],
                    rhs=atn[:, mt, ts(nn, 512)],
                    start=(mt == 0), stop=(mt == NT - 1),
                )
        stg = smp.tile([65, 2, 512], F32, name="stg", tag="stg")
        for nn in range(2):
            # stage PSUM out to SBUF immediately so the accumulator slot
            # frees for the next head; the slow divide chain (reciprocal ->
            # DRAM-bounce partition broadcast -> multiply) runs off SBUF.
            nc.vector.tensor_copy(stg[:, nn, :], po2[nn])
        for nn in range(2):
            nc.vector.reciprocal(rc64[64:65, nn, :], stg[64:65, nn, :])
            nc.sync.dma_start(out=cs_dram[h, nn, :], in_=rc64[64:65, nn, :])
            rbt = smp.tile([64, 512], F32, name="rbt", tag="rbt")
            src = cs_dram[h, nn, :]
            nc.sync.dma_start(
                out=rbt,
                in_=bass.AP(tensor=src.tensor, offset=src.offset,
                            ap=[[0, 64]] + list(src.ap)),
            )
            # even heads land on partitions 0:64 of their aT2 pair-chunk
            # directly; odd heads stage in aT then partition-remap via DMA
            div_dst = (
                aT2[ds(0, 64), h // 2, ts(nn, 512)]
                if h % 2 == 0
                else aT[:, h // 2, ts(nn, 512)]
            )
            nc.vector.tensor_tensor(
                out=div_dst, in0=stg[0:64, nn, :], in1=rbt, op=ALU.mult,
            )
            if h == 0 and dbg:
                dump(f"po_h0_n{nn}", stg[:, nn, :])
                dump(f"rc64_h0_n{nn}", rc64[64:65, nn, :])
                dump(f"rbt_h0_n{nn}", rbt)
        if h == 0:
            dump("atn_h0", atn)
        if h % 2 == 1:
            nc.sync.dma_start(out=aT2[ds(64, 64), h // 2, :], in_=aT[:, h // 2, :])

    # driver: scores-pair 0 starts as soon as its q/k tile exists (ACT
    # starts exp'ing early); v and the next pair's q/k are emitted behind
    # the current pair's scores so PE fills its exp-wait slack with them;
    # out-matmuls run one pair behind. Squares for the LN2 sum-of-squares
    # run on idle GpSimd as chunks finish (last chunk on DVE: tail-critical).
    emit_qk(0)
    prev = emit_scores_pair(0)
    emit_v()
    emit_qk(1)
    for pair in range(1, 4):
        atns = emit_scores_pair(pair)
        if pair < 3:
            emit_qk(pair + 1)
        pp = pair - 1
        emit_out(2 * pp, prev[0])
        emit_out(2 * pp + 1, prev[1])
        nc.gpsimd.tensor_mul(sq_sb[:, pp, :], aT2[:, pp, :], aT2[:, pp, :])
        prev = atns
    emit_out(6, prev[0])
    emit_out(7, prev[1])
    nc.vector.tensor_mul(sq_sb[:, 3, :], aT2[:, 3, :], aT2[:, 3, :])

    dump("qT", qT)
    dump("kT", kT)
    dump("v", v_sb)
    dump("aT2", aT2)

    # ================ Phase D: LN2 stats + output projection ================
    # z[n,dout] = sum_o a[n,o]*toT[o,dout] per n-tile; the LN2 row sums
    # s1[n] = sum_o a, s2[n] = sum_o a^2 come out n-major (as per-partition
    # columns) from N=1 matmuls sharing/reusing the same stationary chunks.
    s1col = ps_o.tile([128, NT], F32, name="s1col", tag="po")
    s2col = ps_o.tile([128, NT], F32, name="s2col", tag="po")
    z_sb = big.tile([128, NT, INNER], BF16)
    for nt in range(NT):
        pz = ps_m.tile([128, INNER], F32, name="pz", tag="mm")
        for ch in range(DC):
            nc.tensor.matmul(
                pz, lhsT=aT2[:, ch, ts(nt, 128)], rhs=toT_sb[:, ch, :],
                start=(ch == 0), stop=(ch == DC - 1),
            )
            nc.tensor.matmul(
                s1col[:, nt : nt + 1], lhsT=aT2[:, ch, ts(nt, 128)], rhs=ones128,
                start=(ch == 0), stop=(ch == DC - 1),
            )
        for ch in range(DC):
            nc.tensor.matmul(
                s2col[:, nt : nt + 1], lhsT=sq_sb[:, ch, ts(nt, 128)], rhs=ones128,
                start=(ch == 0), stop=(ch == DC - 1),
            )
        nc.vector.tensor_copy(z_sb[:, nt, :], pz)

    # mu = s1/512 ; var = s2/512 - mu^2 ; r2 = s_o / sqrt(var + eps_eff)
    mu = lnp.tile([128, NT], F32, name="mu", tag="mu", bufs=1)
    nc.scalar.mul(mu, s1col, 1.0 / INNER)
    es = lnp.tile([128, NT], F32, name="es", tag="es", bufs=1)
    nc.scalar.mul(es, s2col, 1.0 / INNER)
    musq = lnp.tile([128, NT], F32, name="musq", tag="musq", bufs=1)
    nc.vector.tensor_mul(musq, mu, mu)
    var = lnp.tile([128, NT], F32, name="var", tag="var", bufs=1)
    nc.vector.tensor_sub(var, es, musq)
    sd2 = lnp.tile([128, NT], F32, name="sd2", tag="sd2", bufs=1)
    nc.scalar.activation(sd2, var, AF.Ln, bias=eps2, scale=c["inv_so2"])
    r2 = lnp.tile([128, NT], F32, name="r2", tag="r2", bufs=1)
    nc.scalar.activation(r2, sd2, AF.Exp, scale=-0.5)
    r2n = lnp.tile([128, NT], F32, name="r2n", tag="r2n", bufs=1)
    nc.vector.tensor_scalar_mul(r2n, r2, -1.0)
    dump("mu", mu)
    dump("r2", r2)

    # y = (z - mu*W1) * r2 (+ bias_total), fused as u = (W1*mu) - z ; y = u*(-r2)
    for nt in range(NT):
        yt = outp.tile([128, INNER], F32, name="yt", tag="yt")
        nc.vector.scalar_tensor_tensor(
            out=yt, in0=w1b, scalar=mu[:, nt : nt + 1], in1=z_sb[:, nt, :],
            op0=ALU.mult, op1=ALU.subtract,
        )
        nc.vector.tensor_scalar_mul(yt, yt, r2n[:, nt : nt + 1])
        if need_bt:
            nc.vector.tensor_add(yt, yt, btb)
        nc.sync.dma_start(out=y[ts(nt, 128), :], in_=yt)


def _build(c: dict):
    nc = bacc.Bacc("TRN2", target_bir_lowering=False, debug=False, num_devices=B)
    io = {
        "x": nc.dram_tensor("x", [N, D], F32, kind="ExternalInput").ap(),
        "tqT": nc.dram_tensor("tqT", [D, 3 * INNER], BF16, kind="ExternalInput").ap(),
        "toT": nc.dram_tensor("toT", [INNER, INNER], BF16, kind="ExternalInput").ap(),
        "w1u": nc.dram_tensor("w1u", [INNER], F32, kind="ExternalInput").ap(),
        "y": nc.dram_tensor("y", [N, D], F32, kind="ExternalOutput").ap(),
    }
    if c["need_g1"]:
        io["g1v"] = nc.dram_tensor("g1v", [D], F32, kind="ExternalInput").ap()
    if c["need_b1"]:
        io["b1v"] = nc.dram_tensor("b1v", [D], F32, kind="ExternalInput").ap()
    if c["need_bt"]:
        io["btv"] = nc.dram_tensor("btv", [INNER], F32, kind="ExternalInput").ap()
    reps = c.get("body_reps", 1)
    with tile.TileContext(nc) as tc:
        for r in range(reps):
            with ExitStack() as ctx:
                _emit(ctx, tc, io, c, sfx="" if r == 0 else f"_r{r}")

    nc.compile()

    # The act-table-load pass greedily picks the first set containing each
    # function, thrashing between `natural_log` (Ln) and `exp_and_others`
    # (Exp) on every rstd computation (18 reloads @ ~1.3-2.7us each). All
    # activation funcs this kernel uses (Ln, Exp, Copy, Identity) live
    # together in `natural_log_exp_and_others`, so rewrite the first load to
    # that set and drop the rest.
    from concourse.hw_specs import get_activation_tables
    tset = list(get_activation_tables(nc.m.arch).keys())
    nle = tset.index("natural_log_exp_and_others")
    for blk in nc.main_func.blocks:
        keep, first = [], False
        for inst in blk.instructions:
            if type(inst).__name__ == "InstLoadActFuncSet":
                si = getattr(inst, "sync_info", None)
                clean = si is None or (not si.on_wait and not si.on_update)
                if not first:
                    inst.act_func_set_id = nle
                    first = True
                    keep.append(inst)
                elif not clean:
                    inst.act_func_set_id = nle
                    keep.append(inst)
            else:
                keep.append(inst)
        blk.instructions[:] = keep
    return nc


def _prep(inputs):
    g1 = np.asarray(inputs["g1"], np.float32)
    b1 = np.asarray(inputs["b1"], np.float32)
    g2 = np.asarray(inputs["g2"], np.float32)
    b2 = np.asarray(inputs["b2"], np.float32)
    b_out = np.asarray(inputs["b_out"], np.float32)

    Tq, s_q = _ternary(inputs["W_qkv"])   # [3*inner, d]
    To, s_o = _ternary(inputs["W_out"])   # [dout, o]

    Wp = To * g2[None, :]                 # fold g2 (exact when g2 == 1)
    toT = np.ascontiguousarray(Wp.T)      # [o, dout]
    w1u = Wp.sum(axis=1).astype(np.float32)
    bias_total = (b2 @ To.T) * np.float32(s_o) + b_out

    c = {
        "scale_exp": float(s_q * s_q * (DH ** -0.5)),
        "inv_so2": float(1.0 / (s_o * s_o)),
        "eps_eff": float(EPS_LN / (s_q * s_q * s_o * s_o)),
        "need_g1": bool(not np.allclose(g1, 1.0)),
        "need_b1": bool(np.any(b1)),
        "need_bt": bool(np.any(bias_total)),
    }
    arrs = {
        "tqT": np.ascontiguousarray(Tq.T),
        "toT": toT,
        "w1u": w1u,
        "g1": g1, "b1": b1, "bt": bias_total,
    }
    return c, arrs


def _to_bf16(a):
    import ml_dtypes
    return np.asarray(a, np.float32).astype(ml_dtypes.bfloat16)


def kernel(**inputs) -> np.ndarray:
    global LAST_RESULTS
    x = np.asarray(inputs["x"], np.float32)
    assert x.shape == (B, N, D)
    c, arrs = _prep(inputs)

    key = tuple(sorted(c.items()))
    if key not in _CACHE:
        _CACHE[key] = _build(c)
    nc = _CACHE[key]

    base = {
        "tqT": _to_bf16(arrs["tqT"]),
        "toT": _to_bf16(arrs["toT"]),
        "w1u": arrs["w1u"].astype(np.float32),
    }
    if c["need_g1"]:
        base["g1v"] = arrs["g1"]
    if c["need_b1"]:
        base["b1v"] = arrs["b1"]
    if c["need_bt"]:
        base["btv"] = arrs["bt"].astype(np.float32)

    in_maps = [dict(base, x=np.ascontiguousarray(x[i])) for i in range(B)]
    res = run_bass_kernel_spmd(nc, in_maps, core_ids=list(range(B)), trace=TRACE)
    LAST_RESULTS = res
    out = np.stack([res.results[i]["y"] for i in range(B)], axis=0)
    return out.astype(np.float32)


def bench_exec_ns(inputs, iters=32, reps=5, body_reps=1):
    """Measure per-execution NEFF time by chaining `iters` sequential
    executions inside one jitted program (chained through the output
    buffers) and comparing against a 1-execution program."""
    import time as _time
    import jax
    from jax.experimental.shard_map import shard_map
    from jax.sharding import Mesh, PartitionSpec, NamedSharding
    from concourse import bass2jax, mybir as _mybir

    x = np.asarray(inputs["x"], np.float32)
    c, arrs = _prep(inputs)
    if body_reps != 1:
        c["body_reps"] = body_reps
    key = tuple(sorted(c.items()))
    if key not in _CACHE:
        _CACHE[key] = _build(c)
    nc = _CACHE[key]
    bass2jax.install_neuronx_cc_hook()

    base = {
        "tqT": _to_bf16(arrs["tqT"]),
        "toT": _to_bf16(arrs["toT"]),
        "w1u": arrs["w1u"].astype(np.float32),
    }
    if c["need_g1"]:
        base["g1v"] = arrs["g1"]
    if c["need_b1"]:
        base["b1v"] = arrs["b1"]
    if c["need_bt"]:
        base["btv"] = arrs["bt"].astype(np.float32)
    in_maps = [dict(base, x=np.ascontiguousarray(x[i])) for i in range(B)]

    partition_name = nc.partition_id_tensor.name if nc.partition_id_tensor else None
    in_names, out_names, out_avals, zero_outs = [], [], [], []
    for alloc in nc.m.functions[0].allocations:
        if not isinstance(alloc, mybir.MemoryLocationSet):
            continue
        name = alloc.memorylocations[0].name
        if alloc.kind == "ExternalInput":
            if name != partition_name:
                in_names.append(name)
        elif alloc.kind == "ExternalOutput":
            out_names.append(name)
            shape = tuple(alloc.tensor_shape)
            dtype = mybir.dt.np(alloc.dtype)
            out_avals.append(jax.core.ShapedArray(shape, dtype))
            zero_outs.append(np.zeros(shape, dtype))
    n_params = len(in_names)

    bind_names = list(in_names) + list(out_names)
    if partition_name is not None:
        bind_names.append(partition_name)

    def _body(*args):
        operands = list(args)
        pid = [bass2jax.partition_id_tensor()] if partition_name else []
        outs = bass2jax._bass_exec_p.bind(
            *(operands + pid),
            out_avals=tuple(out_avals),
            in_names=tuple(bind_names),
            out_names=tuple(out_names),
            lowering_input_output_aliases=(),
            sim_require_finite=True,
            sim_require_nnan=True,
            nc=nc,
        )
        return tuple(outs)

    devices = jax.devices()[:B]
    mesh = Mesh(np.asarray(devices), ("core",))
    spec = PartitionSpec("core")
    n_out = len(out_names)
    per_core = [[np.asarray(m[nm]) for nm in in_names] for m in in_maps]
    concat_in = [
        np.concatenate([per_core[cc][i] for cc in range(B)], axis=0)
        for i in range(n_params)
    ]
    concat_zeros = [
        np.zeros((B * z.shape[0], *z.shape[1:]), z.dtype) for z in zero_outs
    ]
    dev_args = [
        jax.device_put(a, NamedSharding(mesh, spec)) for a in concat_in + concat_zeros
    ]

    f = jax.jit(
        shard_map(
            _body, mesh=mesh,
            in_specs=(spec,) * (n_params + n_out),
            out_specs=(spec,) * n_out,
            check_rep=False,
        )
    )
    jax.block_until_ready(f(*dev_args))  # compile + warm

    times = {}
    for k in (1, iters):
        best = float("inf")
        for _ in range(reps):
            t0 = _time.perf_counter()
            r = None
            for _ in range(k):
                r = f(*dev_args)  # async dispatch; device executes in-order
            jax.block_until_ready(r)
            best = min(best, _time.perf_counter() - t0)
        times[k] = best
    exec_ns = (times[iters] - times[1]) / (iters - 1) * 1e9
    return exec_ns, times

